# revision 42
# baseline (speedup 1.0000x reference)
"""Trainium2 Bass kernel for the quirky MultiHeadAttention module.

Reference computation (B=4, S=1024, H=768, NH=12, HS=64):
    Q = (x@Wq+bq)  split into heads     [B,12,S,64]
    K = (x@Wk+bk)  split into heads     [B,12,S,64]
    V = x@Wv+bv    NOT split            [B,S,768]
    A = softmax(QK^T/8 + mask)          [B,12,S,S]
    out = (A @ V) reshaped [B, S*12, H] @ Wo + bo    -> [4, 12288, 768]

Algebraic restructuring:
  * (A @ V) @ Wo = A @ (V @ Wo); with zero-able bias fold (bv@Wo+bo as a
    rank-1 constant row, exact because softmax rows sum to 1 against the
    sigma column below).
  * Masked keys are dropped host-side and the key axis compacted (~2x
    less attention work for a Bernoulli(1/2) mask).
  * Softmax denominator = a ones-column appended to VW; no max-subtract.

Layouts (per core: one batch x 6-head group, pure SPMD, no collectives):
    QT/KT: [384 feat, tok]; S^T = KT_h.T @ QT_h -> [k, q] so the mask is a
    per-partition Exp bias; U = exp(S^T) fp16; O = U.T @ [VW | 1] -> [q,769].

Perf structure (vs the 149.6us baseline):
  * Output written fp16 (151->75MB total) and upcast on host: removes the
    output-DMA backpressure that stalled the PE near the end.
  * Overflow-key row packing: keys beyond 4 full 128-tiles (<=32 of them
    for this data) go in a 32-slot group replicated 4x across partitions;
    the per-head overflow PV matmuls address disjoint PE row groups
    (tile_position) so adjacent heads' overflow MMs run concurrently.
    PV cost ~ 4.1 key tiles instead of 5.
  * Cross-chunk software pipelining: chunk c+1's score MMs + exps are
    sprinkled between chunk c's PV groups, so PV never waits on exp.
  * HAM clock management: dense dummy matmuls from t=0 until real work
    arrives keep the PE activity window busy so the 2.4 GHz clock gate
    opens ~4us in and never re-throttles (idle >3.4us closes it).
  * Input DMA: x in 12 fine pieces on the two HWDGE rings (wq halves
    first), wk+wvp on the gpsimd SWDGE ring; output DMA alternates
    sync/gpsimd rings, one contiguous [128,768] fp16 transfer per combo.
"""

import math

import numpy as np

B, S, H, NH, HS = 4, 1024, 768, 12, 64
GW = 384          # head-group width = 6 heads * 64
NCORES = 8

_PROGRAM_CACHE = {}


def _pack6(a):
    """[768, N] -> partition-major [128, 6*N] (tile i at cols i*N:(i+1)*N)."""
    n = a.shape[1]
    return np.ascontiguousarray(
        a.reshape(6, 128, n).transpose(1, 0, 2).reshape(128, 6 * n))


def _build_program(kt_full, ov, has_cvec):
    """kt_full: number of full 128-wide compacted-key tiles.
    ov: overflow group width (0 = none, 32 or 64); overflow keys live in a
    [128, .] stack replicated 128//ov times so per-head overflow PV MMs can
    target disjoint PE row groups and run concurrently.
    has_cvec: include the rank-1 (bv@Wo + bo) constant row in VW."""
    import concourse.mybir as mybir
    import concourse.tile as tile
    from concourse import bacc
    from concourse.bass import ds, ts

    f32 = mybir.dt.float32
    f16 = mybir.dt.float16
    AF = mybir.ActivationFunctionType

    KF = 128 * kt_full            # full-tile key span
    nt = kt_full + (1 if ov else 0)   # tiles incl. overflow stack
    # key chunks (<=512 wide) of the full span for the KT projection
    kchunks = []
    o = 0
    while o < KF:
        w = min(512, KF - o)
        kchunks.append((o, w))
        o += w

    nc = bacc.Bacc(None, target_bir_lowering=False, debug=False)

    xp_d = nc.dram_tensor("xp", (128, 6 * 1024), f16, kind="ExternalInput")
    wqp_d = nc.dram_tensor("wqp", (128, 6 * 384), f16, kind="ExternalInput")
    wkp_d = nc.dram_tensor("wkp", (128, 6 * 384), f16, kind="ExternalInput")
    wvp_d = nc.dram_tensor("wvp", (128, 6 * 768), f16, kind="ExternalInput")
    wvo6_d = nc.dram_tensor("wvo6", (1, 768), f16, kind="ExternalInput")
    xk5_d = (nc.dram_tensor("xk5", (128, 6 * 128), f16, kind="ExternalInput")
             if ov else None)
    # small fp32 per-partition vectors: cols = bq(3) bk(3) mk(kt_full) mk5(0/1)
    sv_d = nc.dram_tensor("sv", (128, 6 + nt), f32, kind="ExternalInput")
    out_d = nc.dram_tensor("out", (6, 1024, 768), f16, kind="ExternalOutput")

    with tile.TileContext(nc) as tc:
        with (
            tc.tile_pool(name="persist", bufs=1) as pp,
            tc.tile_pool(name="ut", bufs=2 * (2 * kt_full + (1 if ov else 0))) as utp,
            tc.tile_pool(name="eps", bufs=8) as ep,
            tc.tile_pool(name="osb", bufs=4) as op_,
        ):
            # ---- stream inputs into SBUF (order = load priority) ----
            sv = pp.tile([128, 6 + nt], f32, name="sv", tag="sv")
            nc.sync.dma_start(sv[:], sv_d[:])
            bq_t = [sv[:, j:j + 1] for j in range(3)]
            bk_t = [sv[:, 3 + j:4 + j] for j in range(3)]
            mk_t = [sv[:, 6 + k:7 + k] for k in range(kt_full)]
            mk5 = sv[:, 6 + kt_full:7 + kt_full] if ov else None

            xbig = pp.tile([128, 6 * 1024], f16, name="xbig", tag="xbig")
            wqbig = pp.tile([128, 6 * 384], f16, name="wqbig", tag="wqbig")
            wkbig = pp.tile([128, 6 * 384], f16, name="wkbig", tag="wkbig")
            wvbig = pp.tile([128, 6 * 768], f16, name="wvbig", tag="wvbig")
            xk5 = (pp.tile([128, 6 * 128], f16, name="xk5", tag="xk5")
                   if ov else None)
            xkt6 = pp.tile([1, 128], f16, name="xkt6", tag="xkt6")
            wvo6 = pp.tile([1, 768], f16, name="wvo6", tag="wvo6")
            # Input loads: wq halves first on the two HWDGE rings, then x in
            # twelve 512-col pieces (ring = query-half so both halves of a
            # kt tile land together), wk+wvp on the SWDGE ring.
            # Ring budget: each HWDGE ring sustains ~80GB/s, SWDGE ~180+.
            # wq halves lead the HWDGE rings (they gate QT), the key halves
            # (qc0, needed by KT/VW too) follow, the tail of x rides SWDGE
            # together with wk/xk5/wvp in dependency order.
            rings = [nc.sync, nc.scalar]
            wh = 3 * 384
            for r in range(2):
                rings[r].dma_start(wqbig[:, r * wh:(r + 1) * wh],
                                   wqp_d[:, r * wh:(r + 1) * wh])
            for kt in range(4):          # qc1 of kt0-3 on SWDGE
                cs = ds(kt * 1024 + 512, 512)
                nc.gpsimd.dma_start(xbig[:, cs], xp_d[:, cs])
            for kt in (4, 5):            # both halves of kt4-5 ride SWDGE
                cs = ds(kt * 1024, 1024)
                nc.gpsimd.dma_start(xbig[:, cs], xp_d[:, cs])
            for kt in range(4):          # key halves of kt0-3 on HWDGE
                cs = ds(kt * 1024, 512)
                rings[kt % 2].dma_start(xbig[:, cs], xp_d[:, cs])
            for r in range(2):
                rings[r].dma_start(wkbig[:, r * wh:(r + 1) * wh],
                                   wkp_d[:, r * wh:(r + 1) * wh])
            if ov:
                nc.gpsimd.dma_start(xk5[:], xk5_d[:])
            if has_cvec:
                nc.vector.memset(xkt6[:], 1.0)
                nc.sync.dma_start(wvo6[:], wvo6_d[:])
            nc.gpsimd.dma_start(wvbig[:], wvp_d[:])

            xt = [xbig[:, i * 1024:(i + 1) * 1024] for i in range(6)]
            wq_t = [wqbig[:, i * 384:(i + 1) * 384] for i in range(6)]
            # tokens are host-permuted (kept keys first), so the K-side
            # tiles are the leading columns of the same x buffer
            xkt = [xbig[:, i * 1024:i * 1024 + KF] for i in range(6)]
            xk5t = ([xk5[:, i * 128:(i + 1) * 128] for i in range(6)]
                    if ov else None)
            wk_t = [wkbig[:, i * 384:(i + 1) * 384] for i in range(6)]
            wvo_t = [wvbig[:, i * 768:(i + 1) * 768] for i in range(6)]

            # persistent intermediates
            KW = KF + (128 if ov else 0)
            QT = [pp.tile([128, 1024], f16, name=f"QT{j}", tag=f"QT{j}")
                  for j in range(3)]
            KT = [pp.tile([128, KW], f16, name=f"KT{j}", tag=f"KT{j}")
                  for j in range(3)]
            VW = [pp.tile([128, 769], f16, name=f"VW{m}", tag=f"VW{m}")
                  for m in range(nt)]

            # ---- PE warm-up ----
            # Dense dummy matmuls keep the PE activity window busy from t=0
            # so the HAM clock gate opens (~3.4us of sustained activity) and
            # real work runs at 2.4 GHz from the start.
            wsrc = pp.tile([128, 512], f16, name="wsrc", tag="wsrc")
            nc.vector.memset(wsrc[:], 0.0)
            # bufs=1: one PSUM bank, stays open through phase A (PSUM there
            # is qk 6 banks + qk5 1 bank + this = 8 exactly)
            psW_cm = tc.tile_pool(name="psW", bufs=1, space="PSUM")
            psW = psW_cm.__enter__()

            def warm(n):
                # dummy matmuls: keep the PE activity window busy (HAM gate)
                # while waiting on input DMA; free when the PE would idle
                for _ in range(n):
                    psw = psW.tile([1, 512], f32, name="warm", tag="warm")
                    nc.tensor.matmul(psw[:], wsrc[:, 0:1], wsrc[:])

            warm(14)

            # ---- phase A: projections ----
            with tc.tile_pool(name="psA", bufs=6, space="PSUM") as psA:
                # QT kt-major: all six (j,qc) PSUM groups accumulate in
                # parallel so each arriving x piece is consumed immediately.
                qgroups = [(j, qc) for qc in (1, 0) for j in range(3)]
                qps = [psA.tile([128, 512], f32, name=f"qtp{j}{qc}", tag="qk")
                       for j, qc in qgroups]
                for kt in range(6):
                    if kt >= 4:
                        warm(4)   # bridge late-arriving kt4/kt5 pieces
                    for gi, (j, qc) in enumerate(qgroups):
                        nc.tensor.matmul(
                            qps[gi][:], wq_t[kt][:, ts(j, 128)],
                            xt[kt][:, ds(qc * 512, 512)],
                            start=(kt == 0), stop=(kt == 5))
                for gi, (j, qc) in enumerate(qgroups):
                    nc.scalar.activation(
                        QT[j][:, ds(qc * 512, 512)], qps[gi][:], AF.Identity,
                        bias=bq_t[j])
                for j in range(3):
                    for o, w in kchunks:
                        kch = ds(o, w)
                        ps2 = psA.tile([128, 512], f32, name="ktp", tag="qk")
                        for kt in range(6):
                            nc.tensor.matmul(
                                ps2[:, 0:w], wk_t[kt][:, ts(j, 128)],
                                xkt[kt][:, kch],
                                start=(kt == 0), stop=(kt == 5))
                        nc.scalar.activation(
                            KT[j][:, kch], ps2[:, 0:w], AF.Identity,
                            bias=bk_t[j])
                    if ov:
                        ps2 = psA.tile([128, 128], f32, name="ktp5",
                                       tag="qk5", bufs=1)
                        for kt in range(6):
                            nc.tensor.matmul(
                                ps2[:], wk_t[kt][:, ts(j, 128)], xk5t[kt][:],
                                start=(kt == 0), stop=(kt == 5))
                        nc.scalar.activation(
                            KT[j][:, ds(KF, 128)], ps2[:], AF.Identity,
                            bias=bk_t[j])

            psW_cm.__exit__(None, None, None)

            # scores for the first chunk are emitted before VW so their exps
            # overlap the VW matmuls (cross-chunk pipeline warm-up); VW then
            # runs, then the PV loop with next-chunk scores sprinkled in.
            chunks = [(j, qc) for j in range(3) for qc in range(2)]
            nrep = (128 // ov) if ov else 0

            with tc.tile_pool(name="psS", bufs=3, space="PSUM") as psSp:
                def emit_score_unit(ci, kt):
                    """One (kt) pair: both heads' score MM + exp. The two MMs
                    address PE rows 0-63 / 64-127 so they run concurrently."""
                    j, qc = chunks[ci]
                    qch = ds(qc * 512, 512)
                    out = []
                    for hh in range(2):
                        p0 = hh * 64
                        ps = psSp.tile([128, 512], f32, name="psS", tag="psS")
                        nc.tensor.matmul(
                            ps[:], KT[j][p0:p0 + 64, ts(kt, 128)],
                            QT[j][p0:p0 + 64, qch])
                        u = utp.tile([128, 512], f16, name="ut", tag="ut")
                        nc.scalar.activation(u[:], ps[:], AF.Exp,
                                             bias=mk_t[kt])
                        out.append(u)
                    return out

                # Overflow-tile PE row-group per (half, hh): with ov=32 the
                # x replication gives 4 identical key groups, so each head's
                # exp is written to TWO replica row-groups and all four t5
                # MMs of a PV group (pb/pa x head A/B) run concurrently.
                if ov == 32:
                    t5base = {("b", 0): 0, ("b", 1): 32,
                              ("a", 0): 64, ("a", 1): 96}
                elif ov == 64:
                    t5base = {("b", 0): 0, ("b", 1): 64,
                              ("a", 0): 0, ("a", 1): 64}
                else:
                    t5base = {}

                def emit_score_unit5(ci):
                    """Overflow scores: stationary [64, 128-replica-cols];
                    exp into every replica row-group the PV MMs will read."""
                    j, qc = chunks[ci]
                    qch = ds(qc * 512, 512)
                    u5 = utp.tile([128, 512], f16, name="ut5", tag="ut5")
                    for hh in range(2):
                        p0 = hh * 64
                        ps = psSp.tile([128, 512], f32, name="psS5", tag="psS")
                        nc.tensor.matmul(
                            ps[:], KT[j][p0:p0 + 64, ds(KF, 128)],
                            QT[j][p0:p0 + 64, qch])
                        bases = {t5base[h, hh] for h in ("b", "a")}
                        for base in bases:
                            nc.scalar.activation(
                                u5[base:base + ov, :], ps[base:base + ov, :],
                                AF.Exp, bias=mk5[base:base + ov, :])
                    return u5

                def emit_scores(ci):
                    ut = [emit_score_unit(ci, kt) for kt in range(kt_full)]
                    u5 = emit_score_unit5(ci) if ov else None
                    return (ut, u5)

                score_tiles = [None] * 6
                score_tiles[0] = emit_scores(0)

                # ---- VW = x_kept @ (Wv@Wo), sigma ones-column appended ----
                # psV nests inside psS and closes before psO opens: PSUM is
                # psS(8KB) + max(psV 4KB, psO 8KB) = 16KB exactly.
                with tc.tile_pool(name="psV", bufs=2, space="PSUM") as psV:
                    for m in range(nt):
                        xsrc = (xk5t if (ov and m == kt_full) else
                                [xkt[kt][:, ts(m, 128)] for kt in range(6)])
                        for ncn in range(2):
                            fch = ds(ncn * 384, 384)
                            ps = psV.tile([128, 384], f32, name="vw", tag="vw")
                            for kt in range(6):
                                src = xsrc[kt] if isinstance(xsrc, list) \
                                    else xsrc[kt]
                                nc.tensor.matmul(
                                    ps[:], src, wvo_t[kt][:, fch],
                                    start=(kt == 0),
                                    stop=(kt == 5 and not has_cvec))
                            if has_cvec:
                                nc.tensor.matmul(
                                    ps[:], xkt6[:], wvo6[:, fch],
                                    start=False, stop=True)
                            nc.vector.tensor_copy(VW[m][:, fch], ps[:])
                        nc.vector.memset(VW[m][:, 768:769], 1.0)

                # ---- attention main loop (pipelined) ----
                # SWDGE (gpsimd) moves output ~2x faster than a HWDGE ring:
                # give it half the transfers so no ring backlogs at the end
                out_rings = [nc.sync, nc.gpsimd]
                tail_rings = [nc.gpsimd, nc.sync, nc.scalar]
                ring_i = [0]

                fcols = {"b": ds(384, 385), "a": ds(0, 384)}

                def emit_pv_group(ci, mq, ut, u5):
                    """One 128-query block, both heads (A,B) interleaved; the
                    four overflow MMs sit adjacent at the end and address
                    disjoint PE row groups, so they execute concurrently."""
                    j, qc = chunks[ci]
                    mqs = ts(mq, 128)
                    ps = {}
                    for half in ("b", "a"):
                        w = 385 if half == "b" else 384
                        for hh in range(2):
                            # pb gets 3 bufs: the first MM of each group is a
                            # pb chain start, so pb recycling is the critical
                            # path (pa chains start mid-group with more slack)
                            p = psOp.tile([128, w], f32, name=f"ps{half}{hh}",
                                          tag=f"ps{half}",
                                          bufs=(3 if half == "b" else 2))
                            ps[half, hh] = p
                            for kt in range(kt_full):
                                nc.tensor.matmul(
                                    p[:], ut[kt][hh][:, mqs],
                                    VW[kt][:, fcols[half]],
                                    start=(kt == 0),
                                    stop=(kt == kt_full - 1 and not ov))
                    if ov:
                        for half in ("b", "a"):
                            for hh in range(2):
                                base = t5base[half, hh]
                                nc.tensor.matmul(
                                    ps[half, hh][:],
                                    u5[base:base + ov, mqs],
                                    VW[kt_full][base:base + ov, fcols[half]],
                                    start=False, stop=True,
                                    tile_position=(base, 0))
                    obs = {}
                    # pb psums are on the recycle critical path: reciprocal +
                    # pb-scale first for both heads, pa scales after
                    for hh in range(2):
                        rv = ep.tile([128, 1], f32, name="rinv", tag="rinv")
                        nc.vector.reciprocal(rv[:], ps["b", hh][:, 384:385])
                        ps["rv", hh] = rv
                        ob = op_.tile([128, 768], f16, name="ob", tag="ob")
                        obs[hh] = ob
                        nc.vector.tensor_scalar_mul(
                            ob[:, 384:768], ps["b", hh][:, 0:384], rv[:])
                    for hh in range(2):
                        head = j * 2 + hh
                        ob = obs[hh]
                        nc.vector.tensor_scalar_mul(
                            ob[:, 0:384], ps["a", hh][:], ps["rv", hh][:])
                        orow = out_d[head, ds(qc * 512 + mq * 128, 128), :]
                        if ci == 5:
                            # drain the final transfers across three rings in
                            # halves so the end-of-kernel DMA tail is short
                            r0 = tail_rings[ring_i[0] % 3]
                            r1 = tail_rings[(ring_i[0] + 1) % 3]
                            r0.dma_start(orow[:, 0:384], ob[:, 0:384])
                            r1.dma_start(orow[:, 384:768], ob[:, 384:768])
                        else:
                            out_rings[ring_i[0] % len(out_rings)].dma_start(
                                orow[:], ob[:])
                        ring_i[0] += 1

                # sprinkle schedule: during chunk c's 4 PV groups, emit chunk
                # c+1's score units (kt_full full pairs + 1 overflow pair).
                with tc.tile_pool(name="psO", bufs=2, space="PSUM") as psOp:
                    for ci in range(6):
                        ut, u5 = score_tiles[ci]
                        nunits = kt_full + (1 if ov else 0)
                        nxt = ([], None)
                        for mq in range(4):
                            emit_pv_group(ci, mq, ut, u5)
                            if ci + 1 < 6:
                                lo = (nunits * mq) // 4
                                hi = (nunits * (mq + 1)) // 4
                                for unit in range(lo, hi):
                                    if unit < kt_full:
                                        nxt[0].append(
                                            emit_score_unit(ci + 1, unit))
                                    else:
                                        nxt = (nxt[0],
                                               emit_score_unit5(ci + 1))
                        if ci + 1 < 6:
                            score_tiles[ci + 1] = nxt
    nc.compile()
    return nc


def get_program(kt_full, ov, has_cvec):
    key = (kt_full, ov, has_cvec)
    if key not in _PROGRAM_CACHE:
        _PROGRAM_CACHE[key] = _build_program(*key)
    return _PROGRAM_CACHE[key]


def prep(x, mask, Wq, bq, Wk, bk, Wv, bv, Wo, bo):
    """Host-side sharding/compaction.
    Tokens are permuted per batch so unmasked keys come first; the device
    computes in permuted token order and gather_output undoes it."""
    f16 = np.float16
    x = np.asarray(x, np.float32)
    mask = np.asarray(mask)
    Wq = np.asarray(Wq, np.float32)
    Wk = np.asarray(Wk, np.float32)
    Wv = np.asarray(Wv, np.float32)
    Wo = np.asarray(Wo, np.float32)
    bq = np.asarray(bq, np.float32)
    bk = np.asarray(bk, np.float32)
    bv = np.asarray(bv, np.float32)
    bo = np.asarray(bo, np.float32)

    mrow = [mask[b, 0, 0] != 0 for b in range(B)]
    perms = [np.argsort(~mrow[b], kind="stable") for b in range(B)]
    nkeep = [int(mrow[b].sum()) for b in range(B)]
    nkmax = max(1, max(nkeep))
    kt_full, r = divmod(nkmax, 128)
    if kt_full == 0:
        kt_full, r = 1, 0
    if r == 0:
        ov = 0
    elif r <= 32:
        ov = 32
    elif r <= 64:
        ov = 64
    else:
        kt_full, ov = kt_full + 1, 0
    KF = 128 * kt_full
    nrep = (128 // ov) if ov else 0
    nt = kt_full + (1 if ov else 0)

    cvec = bv @ Wo + bo
    has_cvec = bool(np.any(cvec))

    # per-head-group packed weights (shared across the 4 batches)
    wq_p, wk_p, bq_p, bk_p = [], [], [], []
    for g in range(2):
        cs = slice(g * GW, (g + 1) * GW)
        wq_p.append(_pack6((Wq[:, cs] * 0.125).astype(f16)))
        wk_p.append(_pack6(Wk[:, cs].astype(f16)))
        bq_p.append((bq[cs] * 0.125).reshape(3, 128).T)   # [128,3]
        bk_p.append(bk[cs].reshape(3, 128).T)
    wvp = _pack6((Wv @ Wo).astype(f16))
    wvo6 = cvec.astype(f16).reshape(1, 768)

    xp_b, xk5_b, sv_b = [], [], []
    for b in range(B):
        xpm = x[b][perms[b]].T.astype(f16)        # [768, 1024] permuted
        xp_b.append(_pack6(xpm))
        sv = np.empty((128, 6 + nt), np.float32)
        mkf = np.full(KF, -1e9, np.float32)
        mkf[:min(nkeep[b], KF)] = 0.0
        sv[:, 6:6 + kt_full] = mkf.reshape(kt_full, 128).T
        if ov:
            g = xpm[:, KF:KF + ov]                # [768, ov] overflow tokens
            xk5_b.append(_pack6(np.ascontiguousarray(np.tile(g, (1, nrep)))))
            nk5 = min(max(nkeep[b] - KF, 0), ov)
            m5 = np.full(ov, -1e9, np.float32)
            m5[:nk5] = 0.0
            sv[:, 6 + kt_full] = np.tile(m5, nrep)
        else:
            xk5_b.append(None)
        sv_b.append(sv)

    in_maps = []
    for c in range(NCORES):
        b, g = c // 2, c % 2
        sv = sv_b[b].copy()
        sv[:, 0:3] = bq_p[g]
        sv[:, 3:6] = bk_p[g]
        im = {
            "xp": xp_b[b],
            "wqp": wq_p[g],
            "wkp": wk_p[g],
            "wvp": wvp,
            "wvo6": wvo6,
            "sv": sv,
        }
        if ov:
            im["xk5"] = xk5_b[b]
        in_maps.append(im)
    return (kt_full, ov, has_cvec), in_maps, perms


def gather_output(results, perms):
    out = np.empty((B, S * NH, H), np.float32)
    ov = out.reshape(B, S, NH, H)
    for c in range(NCORES):
        b, g = c // 2, c % 2
        o = results[c]["out"]  # [6, 1024(permuted q), 768] fp16
        ov[b, perms[b], g * 6:(g + 1) * 6, :] = \
            o.transpose(1, 0, 2).astype(np.float32)
    return out


def kernel(**inputs):
    from concourse.bass_utils import run_bass_kernel_spmd

    cfg, in_maps, perms = prep(**inputs)
    nc = get_program(*cfg)
    res = run_bass_kernel_spmd(nc, in_maps, core_ids=list(range(NCORES)))
    return gather_output(res.results, perms)


if __name__ == "__main__":
    rng = np.random.default_rng(0)
    demo = {
        "x": rng.standard_normal((B, S, H), dtype=np.float32),
        "mask": rng.integers(0, 2, (B, 1, 1, S)).astype(np.int32),
        "Wq": rng.standard_normal((H, H), dtype=np.float32) / np.sqrt(H),
        "bq": np.zeros(H, np.float32),
        "Wk": rng.standard_normal((H, H), dtype=np.float32) / np.sqrt(H),
        "bk": np.zeros(H, np.float32),
        "Wv": rng.standard_normal((H, H), dtype=np.float32) / np.sqrt(H),
        "bv": np.zeros(H, np.float32),
        "Wo": rng.standard_normal((H, H), dtype=np.float32) / np.sqrt(H),
        "bo": np.zeros(H, np.float32),
    }
    out = kernel(**demo)
    print("kernel ran, output shape", out.shape)


# revision 43
# speedup vs baseline: 1.0349x; 1.0349x over previous
"""Trainium2 Bass kernel for the quirky MultiHeadAttention module.

Reference computation (B=4, S=1024, H=768, NH=12, HS=64):
    Q = (x@Wq+bq)  split into heads     [B,12,S,64]
    K = (x@Wk+bk)  split into heads     [B,12,S,64]
    V = x@Wv+bv    NOT split            [B,S,768]
    A = softmax(QK^T/8 + mask)          [B,12,S,S]
    out = (A @ V) reshaped [B, S*12, H] @ Wo + bo    -> [4, 12288, 768]

Algebraic restructuring:
  * (A @ V) @ Wo = A @ (V @ Wo); with zero-able bias fold (bv@Wo+bo as a
    rank-1 constant row, exact because softmax rows sum to 1 against the
    sigma column below).
  * Masked keys are dropped host-side and the key axis compacted (~2x
    less attention work for a Bernoulli(1/2) mask).
  * Softmax denominator = a ones-column appended to VW; no max-subtract.

Layouts (per core: one batch x 6-head group, pure SPMD, no collectives):
    QT/KT: [384 feat, tok]; S^T = KT_h.T @ QT_h -> [k, q] so the mask is a
    per-partition Exp bias; U = exp(S^T) fp16; O = U.T @ [VW | 1] -> [q,769].

Perf structure (vs the 149.6us baseline):
  * Output written fp16 (151->75MB total) and upcast on host: removes the
    output-DMA backpressure that stalled the PE near the end.
  * Overflow-key row packing: keys beyond 4 full 128-tiles (<=32 of them
    for this data) go in a 32-slot group replicated 4x across partitions;
    the per-head overflow PV matmuls address disjoint PE row groups
    (tile_position) so adjacent heads' overflow MMs run concurrently.
    PV cost ~ 4.1 key tiles instead of 5.
  * Cross-chunk software pipelining: chunk c+1's score MMs + exps are
    sprinkled between chunk c's PV groups, so PV never waits on exp.
  * HAM clock management: dense dummy matmuls from t=0 until real work
    arrives keep the PE activity window busy so the 2.4 GHz clock gate
    opens ~4us in and never re-throttles (idle >3.4us closes it).
  * Input DMA: x in 12 fine pieces on the two HWDGE rings (wq halves
    first), wk+wvp on the gpsimd SWDGE ring; output DMA alternates
    sync/gpsimd rings, one contiguous [128,768] fp16 transfer per combo.
"""

import math

import numpy as np

B, S, H, NH, HS = 4, 1024, 768, 12, 64
GW = 384          # head-group width = 6 heads * 64
NCORES = 8

_PROGRAM_CACHE = {}


def _pack6(a):
    """[768, N] -> partition-major [128, 6*N] (tile i at cols i*N:(i+1)*N)."""
    n = a.shape[1]
    return np.ascontiguousarray(
        a.reshape(6, 128, n).transpose(1, 0, 2).reshape(128, 6 * n))


def _build_program(kt_full, ov, has_cvec):
    """kt_full: number of full 128-wide compacted-key tiles.
    ov: overflow group width (0 = none, 32 or 64); overflow keys live in a
    [128, .] stack replicated 128//ov times so per-head overflow PV MMs can
    target disjoint PE row groups and run concurrently.
    has_cvec: include the rank-1 (bv@Wo + bo) constant row in VW."""
    import concourse.mybir as mybir
    import concourse.tile as tile
    from concourse import bacc
    from concourse.bass import ds, ts

    f32 = mybir.dt.float32
    f16 = mybir.dt.float16
    AF = mybir.ActivationFunctionType

    KF = 128 * kt_full            # full-tile key span
    nt = kt_full + (1 if ov else 0)   # tiles incl. overflow stack
    # key chunks (<=512 wide) of the full span for the KT projection
    kchunks = []
    o = 0
    while o < KF:
        w = min(512, KF - o)
        kchunks.append((o, w))
        o += w

    nc = bacc.Bacc(None, target_bir_lowering=False, debug=False)

    xp_d = nc.dram_tensor("xp", (128, 6 * 1024), f16, kind="ExternalInput")
    wqp_d = nc.dram_tensor("wqp", (128, 6 * 384), f16, kind="ExternalInput")
    wkp_d = nc.dram_tensor("wkp", (128, 6 * 384), f16, kind="ExternalInput")
    wvp_d = nc.dram_tensor("wvp", (128, 6 * 768), f16, kind="ExternalInput")
    wvo6_d = nc.dram_tensor("wvo6", (1, 768), f16, kind="ExternalInput")
    xk5_d = (nc.dram_tensor("xk5", (128, 6 * 128), f16, kind="ExternalInput")
             if ov else None)
    # small fp32 per-partition vectors: cols = bq(3) bk(3) mk(kt_full) mk5(0/1)
    sv_d = nc.dram_tensor("sv", (128, 6 + nt), f32, kind="ExternalInput")
    out_d = nc.dram_tensor("out", (6, 1024, 768), f16, kind="ExternalOutput")

    with tile.TileContext(nc) as tc:
        with (
            tc.tile_pool(name="persist", bufs=1) as pp,
            tc.tile_pool(name="ut", bufs=2 * (2 * kt_full + (1 if ov else 0))) as utp,
            tc.tile_pool(name="eps", bufs=8) as ep,
            tc.tile_pool(name="osb", bufs=4) as op_,
        ):
            # ---- stream inputs into SBUF (order = load priority) ----
            sv = pp.tile([128, 6 + nt], f32, name="sv", tag="sv")
            nc.sync.dma_start(sv[:], sv_d[:])
            bq_t = [sv[:, j:j + 1] for j in range(3)]
            bk_t = [sv[:, 3 + j:4 + j] for j in range(3)]
            mk_t = [sv[:, 6 + k:7 + k] for k in range(kt_full)]
            mk5 = sv[:, 6 + kt_full:7 + kt_full] if ov else None

            xbig = pp.tile([128, 6 * 1024], f16, name="xbig", tag="xbig")
            wqbig = pp.tile([128, 6 * 384], f16, name="wqbig", tag="wqbig")
            wkbig = pp.tile([128, 6 * 384], f16, name="wkbig", tag="wkbig")
            wvbig = pp.tile([128, 6 * 768], f16, name="wvbig", tag="wvbig")
            xk5 = (pp.tile([128, 6 * 128], f16, name="xk5", tag="xk5")
                   if ov else None)
            xkt6 = pp.tile([1, 128], f16, name="xkt6", tag="xkt6")
            wvo6 = pp.tile([1, 768], f16, name="wvo6", tag="wvo6")
            # Input loads: wq halves first on the two HWDGE rings, then x in
            # twelve 512-col pieces (ring = query-half so both halves of a
            # kt tile land together), wk+wvp on the SWDGE ring.
            # Ring budget: each HWDGE ring sustains ~80GB/s, SWDGE ~180+.
            # wq halves lead the HWDGE rings (they gate QT), the key halves
            # (qc0, needed by KT/VW too) follow, the tail of x rides SWDGE
            # together with wk/xk5/wvp in dependency order.
            rings = [nc.sync, nc.scalar]
            wh = 3 * 384
            for r in range(2):
                rings[r].dma_start(wqbig[:, r * wh:(r + 1) * wh],
                                   wqp_d[:, r * wh:(r + 1) * wh])
            for kt in range(4):          # qc1 of kt0-3 on SWDGE
                cs = ds(kt * 1024 + 512, 512)
                nc.gpsimd.dma_start(xbig[:, cs], xp_d[:, cs])
            for kt in (4, 5):            # both halves of kt4-5 ride SWDGE
                cs = ds(kt * 1024, 1024)
                nc.gpsimd.dma_start(xbig[:, cs], xp_d[:, cs])
            for kt in range(4):          # key halves of kt0-3 on HWDGE
                cs = ds(kt * 1024, 512)
                rings[kt % 2].dma_start(xbig[:, cs], xp_d[:, cs])
            for r in range(2):
                rings[r].dma_start(wkbig[:, r * wh:(r + 1) * wh],
                                   wkp_d[:, r * wh:(r + 1) * wh])
            if ov:
                nc.gpsimd.dma_start(xk5[:], xk5_d[:])
            if has_cvec:
                nc.vector.memset(xkt6[:], 1.0)
                nc.sync.dma_start(wvo6[:], wvo6_d[:])
            nc.gpsimd.dma_start(wvbig[:], wvp_d[:])

            xt = [xbig[:, i * 1024:(i + 1) * 1024] for i in range(6)]
            wq_t = [wqbig[:, i * 384:(i + 1) * 384] for i in range(6)]
            # tokens are host-permuted (kept keys first), so the K-side
            # tiles are the leading columns of the same x buffer
            xkt = [xbig[:, i * 1024:i * 1024 + KF] for i in range(6)]
            xk5t = ([xk5[:, i * 128:(i + 1) * 128] for i in range(6)]
                    if ov else None)
            wk_t = [wkbig[:, i * 384:(i + 1) * 384] for i in range(6)]
            wvo_t = [wvbig[:, i * 768:(i + 1) * 768] for i in range(6)]

            # persistent intermediates
            KW = KF + (128 if ov else 0)
            QT = [pp.tile([128, 1024], f16, name=f"QT{j}", tag=f"QT{j}")
                  for j in range(3)]
            KT = [pp.tile([128, KW], f16, name=f"KT{j}", tag=f"KT{j}")
                  for j in range(3)]
            VW = [pp.tile([128, 769], f16, name=f"VW{m}", tag=f"VW{m}")
                  for m in range(nt)]

            # ---- PE warm-up ----
            # Dense dummy matmuls keep the PE activity window busy from t=0
            # so the HAM clock gate opens (~3.4us of sustained activity) and
            # real work runs at 2.4 GHz from the start.
            wsrc = pp.tile([128, 512], f16, name="wsrc", tag="wsrc")
            nc.vector.memset(wsrc[:], 0.0)
            # bufs=1: one PSUM bank, stays open through phase A (PSUM there
            # is qk 6 banks + qk5 1 bank + this = 8 exactly)
            psW_cm = tc.tile_pool(name="psW", bufs=1, space="PSUM")
            psW = psW_cm.__enter__()

            def warm(n):
                # dummy matmuls: keep the PE activity window busy (HAM gate)
                # while waiting on input DMA; free when the PE would idle.
                # One accumulation group so they stream back-to-back.
                psw = psW.tile([1, 512], f32, name="warm", tag="warm")
                for i in range(n):
                    nc.tensor.matmul(psw[:], wsrc[:, 0:1], wsrc[:],
                                     start=(i == 0), stop=(i == n - 1))

            warm(14)

            # ---- phase A: projections ----
            with tc.tile_pool(name="psA", bufs=6, space="PSUM") as psA:
                # QT kt-major: all six (j,qc) PSUM groups accumulate in
                # parallel so each arriving x piece is consumed immediately.
                qgroups = [(j, qc) for qc in (1, 0) for j in range(3)]
                qps = [psA.tile([128, 512], f32, name=f"qtp{j}{qc}", tag="qk")
                       for j, qc in qgroups]
                for kt in range(6):
                    if kt >= 4:
                        warm(4)   # bridge late-arriving kt4/kt5 pieces
                    for gi, (j, qc) in enumerate(qgroups):
                        nc.tensor.matmul(
                            qps[gi][:], wq_t[kt][:, ts(j, 128)],
                            xt[kt][:, ds(qc * 512, 512)],
                            start=(kt == 0), stop=(kt == 5))
                for gi, (j, qc) in enumerate(qgroups):
                    nc.scalar.activation(
                        QT[j][:, ds(qc * 512, 512)], qps[gi][:], AF.Identity,
                        bias=bq_t[j])
                for j in range(3):
                    for o, w in kchunks:
                        kch = ds(o, w)
                        ps2 = psA.tile([128, 512], f32, name="ktp", tag="qk")
                        for kt in range(6):
                            nc.tensor.matmul(
                                ps2[:, 0:w], wk_t[kt][:, ts(j, 128)],
                                xkt[kt][:, kch],
                                start=(kt == 0), stop=(kt == 5))
                        nc.scalar.activation(
                            KT[j][:, kch], ps2[:, 0:w], AF.Identity,
                            bias=bk_t[j])
                    if ov:
                        ps2 = psA.tile([128, 128], f32, name="ktp5",
                                       tag="qk5", bufs=1)
                        for kt in range(6):
                            nc.tensor.matmul(
                                ps2[:], wk_t[kt][:, ts(j, 128)], xk5t[kt][:],
                                start=(kt == 0), stop=(kt == 5))
                        nc.scalar.activation(
                            KT[j][:, ds(KF, 128)], ps2[:], AF.Identity,
                            bias=bk_t[j])

            psW_cm.__exit__(None, None, None)

            # scores for the first chunk are emitted before VW so their exps
            # overlap the VW matmuls (cross-chunk pipeline warm-up); VW then
            # runs, then the PV loop with next-chunk scores sprinkled in.
            chunks = [(j, qc) for j in range(3) for qc in range(2)]
            nrep = (128 // ov) if ov else 0

            with tc.tile_pool(name="psS", bufs=3, space="PSUM") as psSp:
                def emit_score_unit(ci, kt):
                    """One (kt) pair: both heads' score MM + exp. The two MMs
                    address PE rows 0-63 / 64-127 so they run concurrently."""
                    j, qc = chunks[ci]
                    qch = ds(qc * 512, 512)
                    out = []
                    for hh in range(2):
                        p0 = hh * 64
                        ps = psSp.tile([128, 512], f32, name="psS", tag="psS")
                        nc.tensor.matmul(
                            ps[:], KT[j][p0:p0 + 64, ts(kt, 128)],
                            QT[j][p0:p0 + 64, qch])
                        u = utp.tile([128, 512], f16, name="ut", tag="ut")
                        nc.scalar.activation(u[:], ps[:], AF.Exp,
                                             bias=mk_t[kt])
                        out.append(u)
                    return out

                # Overflow-tile PE row-group per (half, hh): with ov=32 the
                # x replication gives 4 identical key groups, so each head's
                # exp is written to TWO replica row-groups and all four t5
                # MMs of a PV group (pb/pa x head A/B) run concurrently.
                if ov == 32:
                    t5base = {("b", 0): 0, ("b", 1): 32,
                              ("a", 0): 64, ("a", 1): 96}
                elif ov == 64:
                    t5base = {("b", 0): 0, ("b", 1): 64,
                              ("a", 0): 0, ("a", 1): 64}
                else:
                    t5base = {}

                def emit_score_unit5(ci):
                    """Overflow scores: stationary [64, 128-replica-cols];
                    exp into every replica row-group the PV MMs will read."""
                    j, qc = chunks[ci]
                    qch = ds(qc * 512, 512)
                    u5 = utp.tile([128, 512], f16, name="ut5", tag="ut5")
                    for hh in range(2):
                        p0 = hh * 64
                        ps = psSp.tile([128, 512], f32, name="psS5", tag="psS")
                        nc.tensor.matmul(
                            ps[:], KT[j][p0:p0 + 64, ds(KF, 128)],
                            QT[j][p0:p0 + 64, qch])
                        bases = {t5base[h, hh] for h in ("b", "a")}
                        for base in bases:
                            nc.scalar.activation(
                                u5[base:base + ov, :], ps[base:base + ov, :],
                                AF.Exp, bias=mk5[base:base + ov, :])
                    return u5

                def emit_scores(ci):
                    ut = [emit_score_unit(ci, kt) for kt in range(kt_full)]
                    u5 = emit_score_unit5(ci) if ov else None
                    return (ut, u5)

                score_tiles = [None] * 6
                score_tiles[0] = emit_scores(0)

                # ---- VW = x_kept @ (Wv@Wo), sigma ones-column appended ----
                # psV nests inside psS and closes before psO opens: PSUM is
                # psS(8KB) + max(psV 4KB, psO 8KB) = 16KB exactly.
                with tc.tile_pool(name="psV", bufs=2, space="PSUM") as psV:
                    for m in range(nt):
                        xsrc = (xk5t if (ov and m == kt_full) else
                                [xkt[kt][:, ts(m, 128)] for kt in range(6)])
                        for ncn in range(2):
                            fch = ds(ncn * 384, 384)
                            ps = psV.tile([128, 384], f32, name="vw", tag="vw")
                            for kt in range(6):
                                src = xsrc[kt] if isinstance(xsrc, list) \
                                    else xsrc[kt]
                                nc.tensor.matmul(
                                    ps[:], src, wvo_t[kt][:, fch],
                                    start=(kt == 0),
                                    stop=(kt == 5 and not has_cvec))
                            if has_cvec:
                                nc.tensor.matmul(
                                    ps[:], xkt6[:], wvo6[:, fch],
                                    start=False, stop=True)
                            nc.vector.tensor_copy(VW[m][:, fch], ps[:])
                        nc.vector.memset(VW[m][:, 768:769], 1.0)

                # ---- attention main loop (pipelined) ----
                # SWDGE (gpsimd) moves output ~2x faster than a HWDGE ring:
                # give it half the transfers so no ring backlogs at the end
                out_rings = [nc.sync, nc.gpsimd]
                tail_rings = [nc.gpsimd, nc.sync, nc.scalar]
                ring_i = [0]

                fcols = {"b": ds(384, 385), "a": ds(0, 384)}

                def emit_pv_group(ci, mq, ut, u5):
                    """One 128-query block, both heads (A,B) interleaved; the
                    four overflow MMs sit adjacent at the end and address
                    disjoint PE row groups, so they execute concurrently."""
                    j, qc = chunks[ci]
                    mqs = ts(mq, 128)
                    ps = {}
                    for half in ("b", "a"):
                        w = 385 if half == "b" else 384
                        for hh in range(2):
                            # pb gets 3 bufs: the first MM of each group is a
                            # pb chain start, so pb recycling is the critical
                            # path (pa chains start mid-group with more slack)
                            p = psOp.tile([128, w], f32, name=f"ps{half}{hh}",
                                          tag=f"ps{half}",
                                          bufs=(3 if half == "b" else 2))
                            ps[half, hh] = p
                            for kt in range(kt_full):
                                nc.tensor.matmul(
                                    p[:], ut[kt][hh][:, mqs],
                                    VW[kt][:, fcols[half]],
                                    start=(kt == 0),
                                    stop=(kt == kt_full - 1 and not ov))
                    if ov:
                        for half in ("b", "a"):
                            for hh in range(2):
                                base = t5base[half, hh]
                                nc.tensor.matmul(
                                    ps[half, hh][:],
                                    u5[base:base + ov, mqs],
                                    VW[kt_full][base:base + ov, fcols[half]],
                                    start=False, stop=True,
                                    tile_position=(base, 0))
                    obs = {}
                    # pb psums are on the recycle critical path: reciprocal +
                    # pb-scale first for both heads, pa scales after
                    for hh in range(2):
                        rv = ep.tile([128, 1], f32, name="rinv", tag="rinv")
                        nc.vector.reciprocal(rv[:], ps["b", hh][:, 384:385])
                        ps["rv", hh] = rv
                        ob = op_.tile([128, 768], f16, name="ob", tag="ob")
                        obs[hh] = ob
                        nc.vector.tensor_scalar_mul(
                            ob[:, 384:768], ps["b", hh][:, 0:384], rv[:])
                    for hh in range(2):
                        head = j * 2 + hh
                        ob = obs[hh]
                        nc.vector.tensor_scalar_mul(
                            ob[:, 0:384], ps["a", hh][:], ps["rv", hh][:])
                        orow = out_d[head, ds(qc * 512 + mq * 128, 128), :]
                        if ci == 5:
                            # drain the final transfers across three rings in
                            # halves so the end-of-kernel DMA tail is short
                            r0 = tail_rings[ring_i[0] % 3]
                            r1 = tail_rings[(ring_i[0] + 1) % 3]
                            r0.dma_start(orow[:, 0:384], ob[:, 0:384])
                            r1.dma_start(orow[:, 384:768], ob[:, 384:768])
                        else:
                            out_rings[ring_i[0] % len(out_rings)].dma_start(
                                orow[:], ob[:])
                        ring_i[0] += 1

                # sprinkle schedule: during chunk c's 4 PV groups, emit chunk
                # c+1's score units (kt_full full pairs + 1 overflow pair).
                with tc.tile_pool(name="psO", bufs=2, space="PSUM") as psOp:
                    for ci in range(6):
                        ut, u5 = score_tiles[ci]
                        nunits = kt_full + (1 if ov else 0)
                        nxt = ([], None)
                        for mq in range(4):
                            emit_pv_group(ci, mq, ut, u5)
                            if ci + 1 < 6:
                                lo = (nunits * mq) // 4
                                hi = (nunits * (mq + 1)) // 4
                                for unit in range(lo, hi):
                                    if unit < kt_full:
                                        nxt[0].append(
                                            emit_score_unit(ci + 1, unit))
                                    else:
                                        nxt = (nxt[0],
                                               emit_score_unit5(ci + 1))
                        if ci + 1 < 6:
                            score_tiles[ci + 1] = nxt
    nc.compile()
    return nc


def get_program(kt_full, ov, has_cvec):
    key = (kt_full, ov, has_cvec)
    if key not in _PROGRAM_CACHE:
        _PROGRAM_CACHE[key] = _build_program(*key)
    return _PROGRAM_CACHE[key]


def prep(x, mask, Wq, bq, Wk, bk, Wv, bv, Wo, bo):
    """Host-side sharding/compaction.
    Tokens are permuted per batch so unmasked keys come first; the device
    computes in permuted token order and gather_output undoes it."""
    f16 = np.float16
    x = np.asarray(x, np.float32)
    mask = np.asarray(mask)
    Wq = np.asarray(Wq, np.float32)
    Wk = np.asarray(Wk, np.float32)
    Wv = np.asarray(Wv, np.float32)
    Wo = np.asarray(Wo, np.float32)
    bq = np.asarray(bq, np.float32)
    bk = np.asarray(bk, np.float32)
    bv = np.asarray(bv, np.float32)
    bo = np.asarray(bo, np.float32)

    mrow = [mask[b, 0, 0] != 0 for b in range(B)]
    perms = [np.argsort(~mrow[b], kind="stable") for b in range(B)]
    nkeep = [int(mrow[b].sum()) for b in range(B)]
    nkmax = max(1, max(nkeep))
    kt_full, r = divmod(nkmax, 128)
    if kt_full == 0:
        kt_full, r = 1, 0
    if r == 0:
        ov = 0
    elif r <= 32:
        ov = 32
    elif r <= 64:
        ov = 64
    else:
        kt_full, ov = kt_full + 1, 0
    KF = 128 * kt_full
    nrep = (128 // ov) if ov else 0
    nt = kt_full + (1 if ov else 0)

    cvec = bv @ Wo + bo
    has_cvec = bool(np.any(cvec))

    # per-head-group packed weights (shared across the 4 batches)
    wq_p, wk_p, bq_p, bk_p = [], [], [], []
    for g in range(2):
        cs = slice(g * GW, (g + 1) * GW)
        wq_p.append(_pack6((Wq[:, cs] * 0.125).astype(f16)))
        wk_p.append(_pack6(Wk[:, cs].astype(f16)))
        bq_p.append((bq[cs] * 0.125).reshape(3, 128).T)   # [128,3]
        bk_p.append(bk[cs].reshape(3, 128).T)
    wvp = _pack6((Wv @ Wo).astype(f16))
    wvo6 = cvec.astype(f16).reshape(1, 768)

    xp_b, xk5_b, sv_b = [], [], []
    for b in range(B):
        xpm = x[b][perms[b]].T.astype(f16)        # [768, 1024] permuted
        xp_b.append(_pack6(xpm))
        sv = np.empty((128, 6 + nt), np.float32)
        mkf = np.full(KF, -1e9, np.float32)
        mkf[:min(nkeep[b], KF)] = 0.0
        sv[:, 6:6 + kt_full] = mkf.reshape(kt_full, 128).T
        if ov:
            g = xpm[:, KF:KF + ov]                # [768, ov] overflow tokens
            xk5_b.append(_pack6(np.ascontiguousarray(np.tile(g, (1, nrep)))))
            nk5 = min(max(nkeep[b] - KF, 0), ov)
            m5 = np.full(ov, -1e9, np.float32)
            m5[:nk5] = 0.0
            sv[:, 6 + kt_full] = np.tile(m5, nrep)
        else:
            xk5_b.append(None)
        sv_b.append(sv)

    in_maps = []
    for c in range(NCORES):
        b, g = c // 2, c % 2
        sv = sv_b[b].copy()
        sv[:, 0:3] = bq_p[g]
        sv[:, 3:6] = bk_p[g]
        im = {
            "xp": xp_b[b],
            "wqp": wq_p[g],
            "wkp": wk_p[g],
            "wvp": wvp,
            "wvo6": wvo6,
            "sv": sv,
        }
        if ov:
            im["xk5"] = xk5_b[b]
        in_maps.append(im)
    return (kt_full, ov, has_cvec), in_maps, perms


def gather_output(results, perms):
    out = np.empty((B, S * NH, H), np.float32)
    ov = out.reshape(B, S, NH, H)
    for c in range(NCORES):
        b, g = c // 2, c % 2
        o = results[c]["out"]  # [6, 1024(permuted q), 768] fp16
        ov[b, perms[b], g * 6:(g + 1) * 6, :] = \
            o.transpose(1, 0, 2).astype(np.float32)
    return out


def kernel(**inputs):
    from concourse.bass_utils import run_bass_kernel_spmd

    cfg, in_maps, perms = prep(**inputs)
    nc = get_program(*cfg)
    res = run_bass_kernel_spmd(nc, in_maps, core_ids=list(range(NCORES)))
    return gather_output(res.results, perms)


if __name__ == "__main__":
    rng = np.random.default_rng(0)
    demo = {
        "x": rng.standard_normal((B, S, H), dtype=np.float32),
        "mask": rng.integers(0, 2, (B, 1, 1, S)).astype(np.int32),
        "Wq": rng.standard_normal((H, H), dtype=np.float32) / np.sqrt(H),
        "bq": np.zeros(H, np.float32),
        "Wk": rng.standard_normal((H, H), dtype=np.float32) / np.sqrt(H),
        "bk": np.zeros(H, np.float32),
        "Wv": rng.standard_normal((H, H), dtype=np.float32) / np.sqrt(H),
        "bv": np.zeros(H, np.float32),
        "Wo": rng.standard_normal((H, H), dtype=np.float32) / np.sqrt(H),
        "bo": np.zeros(H, np.float32),
    }
    out = kernel(**demo)
    print("kernel ran, output shape", out.shape)
